# revision 18
# baseline (speedup 1.0000x reference)
"""Trainium2 Bass kernel for nn_AdvancedHybridGNN (hybrid GCN+GAT, N=30000, E=600000).

Strategy (8 NeuronCores, graph/data parallel; V6):
- Nodes padded to 30720 = 8 cores x 30 blocks x 128. Real edges sorted by
  destination block and SPLIT BY SOURCE HALF (global table rows for blocks 0-14
  of every core = tblA, blocks 15-29 = tblB), each half padded per block to
  T_half tiles of 128. Self-loops are one extra "self tile" per block loaded
  from the local pre-AllGather buffer with dma_start (no gather descriptors).
- Per layer the table (row = [gcn(128) | xh(128) | A=exp(asrc)(h) |
  A'=exp(.2 asrc)(h) | pad], 384 bf16 cols = 768B) is AllGathered into TWO
  per-half Shared DRAM tensors, each written by exactly one collective. A round
  runs in two phases:
    phase A: per block gather half-0 source rows (+ self tile), weight, and
      aggregate via one-hot S matmuls; evict partial sums to SBUF (bf16).
      Only needs tblA (AllGather fired mid-previous-round), so it overlaps the
      in-flight half-1 AllGather.
    phase B: gather half-1 rows, aggregate, add phase-A partials, epilogues,
      build next layer's table rows, fire the next AllGathers (half-0
      mid-phase, half-1 at the end). Round 2 runs the output heads here.
- GAT softmax factored exactly: dividing num and denom by exp(.2 adst) gives
  per-edge weight ex = max(A * C_dst, A') with C = exp(.8 adst) a per-dst-block
  [128, h] local tile, expanded per edge with a tiny matmul lhsT=S2 (dst-major
  one-hot built on DVE from int8 replicated dstloc rows).
- Layer-0 table built directly from x via host-fused [x;1] weights.
- GCN sym-norm folded as dis[src] into the table and dis[dst] into the
  epilogue; eval BatchNorms folded into per-feature affines; bf16 on SBUF.
"""
import numpy as np
import sys

sys.path.insert(0, "/opt/trn_rl_repo")

import concourse.bacc as bacc
import concourse.bass as bass
import concourse.mybir as mybir
import concourse.tile as tile
from concourse import library_config
from concourse.masks import make_identity
from concourse.bass_utils import run_bass_kernel_spmd

F32 = mybir.dt.float32
BF = mybir.dt.bfloat16
I16 = mybir.dt.int16
I8 = mybir.dt.int8
NPBF = mybir.dt.np(BF)
AF = mybir.ActivationFunctionType
ALU = mybir.AluOpType

N = 30000
NPAD = 30720
E = 600000
IN = 64
HID = 128
EPS = 1e-5
HEADS = (4, 4, 1)
NC = 8
BLK = 128
NBLK = NPAD // BLK      # 240
BPC = NBLK // NC        # 30 blocks per core
TW = 384                # table row width (768 B in bf16; must be %256B)


# ----------------------------------------------------------------------------
# Host-side graph preprocessing
# ----------------------------------------------------------------------------

def preprocess_graph(edge_index):
    src = edge_index[0].astype(np.int64)
    dst = edge_index[1].astype(np.int64)
    # degree includes the self loop (reference adds them)
    deg = np.bincount(dst, minlength=NPAD) + 1
    deg[N:] = 0
    dis = np.zeros(NPAD, np.float32)
    m = deg > 0
    dis[m] = (1.0 / np.sqrt(deg[m].astype(np.float64))).astype(np.float32)

    # permuted-AllGather row id: blocks 0-14 of each core -> tblA, 15-29 -> tblB
    c2 = src // (BPC * BLK)
    k2 = src % (BPC * BLK)
    sh = BPC * BLK // 2
    in_h1 = k2 >= sh
    row = np.where(in_h1, c2 * sh + (k2 - sh), c2 * sh + k2)  # half-relative

    key = dst * 2 + in_h1
    order = np.argsort(key, kind="stable")
    src_r = row[order]
    dst_s = dst[order]
    h1_s = in_h1[order]
    key_s = key[order]
    blk_of = dst_s // BLK
    c0 = np.bincount(blk_of[~h1_s], minlength=NBLK)
    c1 = np.bincount(blk_of[h1_s], minlength=NBLK)
    T_half = max(1, int(np.ceil(max(c0.max(), c1.max()) / 128)))
    EPH = T_half * BLK
    TTT = 2 * T_half + 1
    srcA = np.zeros((NBLK, EPH), np.int64)
    srcB = np.zeros((NBLK, EPH), np.int64)
    # per block tile layout: [h0 (T_half) | self (1) | h1 (T_half)]
    dstloc = np.full((NBLK, TTT * BLK), -1, np.int32)
    dstloc[:, EPH:EPH + BLK] = np.arange(BLK, dtype=np.int32)[None, :]
    for b in range(NBLK):
        lo = np.searchsorted(key_s, (b * BLK) * 2)
        hi = np.searchsorted(key_s, ((b + 1) * BLK) * 2)
        seg_dst = dst_s[lo:hi]
        seg_h1 = h1_s[lo:hi]
        seg_src = src_r[lo:hi]
        m0 = ~seg_h1
        n0 = int(m0.sum())
        n1 = int(len(seg_dst) - n0)
        srcA[b, :n0] = seg_src[m0]
        srcB[b, :n1] = seg_src[seg_h1]
        dstloc[b, :n0] = (seg_dst[m0] - b * BLK).astype(np.int32)
        dstloc[b, EPH + BLK:EPH + BLK + n1] = \
            (seg_dst[seg_h1] - b * BLK).astype(np.int32)
    return (srcA, srcB), None, dstloc, dis, T_half


def wrap_idx_core(idx_tiles):
    """idx_tiles: [T, 128] int -> int16 gather layout [128, T*8]."""
    T = idx_tiles.shape[0]
    a = idx_tiles.reshape(T, 8, 16).transpose(2, 0, 1).reshape(16, T * 8)
    return np.tile(a, (8, 1)).astype(np.int16)


def bn_fold(g):
    return g / np.sqrt(1.0 + EPS)


def make_blockdiag(a, heads):
    C = HID // heads
    bd = np.zeros((HID, heads), np.float32)
    for h in range(heads):
        bd[h * C:(h + 1) * C, h] = a[h * C:(h + 1) * C]
    return bd


# ----------------------------------------------------------------------------
# Device program
# ----------------------------------------------------------------------------

NQ = 4  # SWDGE queues for gather parallelism


def build_program(T_half, ngcn=3):
    """Build the SPMD Bass program (same for all 8 cores)."""
    nc = bacc.Bacc("TRN2", num_swdge_queues=NQ)
    TTA = T_half + 1                    # phase-A tiles (h0 + self)
    TTT = 2 * T_half + 1
    TILES = BPC * T_half                # gathered tiles per half per core
    TILA = BPC * TTT                    # all tiles per core
    HSH = BPC * BLK // 2                # 1920 rows per half-shard

    # ---- inputs ----
    x_fm = nc.dram_tensor("x_fm", [IN + 1, BPC * BLK], BF, kind="ExternalInput")
    idx_a = nc.dram_tensor("idx_a", [128, TILES * 8], I16, kind="ExternalInput")
    idx_b = nc.dram_tensor("idx_b", [128, TILES * 8], I16, kind="ExternalInput")
    dstloc = nc.dram_tensor("dstloc", [128, TILA], BF, kind="ExternalInput")
    dlt_in = nc.dram_tensor("dlt", [128, TILA * 128], BF, kind="ExternalInput")
    iota_rep = nc.dram_tensor("iota_rep", [128, TTA * 128], BF, kind="ExternalInput")
    iotac_in = nc.dram_tensor("iotac", [128, 1], F32, kind="ExternalInput")
    dis_in = nc.dram_tensor("dis", [128, BPC], F32, kind="ExternalInput")

    t0g_in = nc.dram_tensor("t0g", [IN + 1, HID], BF, kind="ExternalInput")
    t0a_in = nc.dram_tensor("t0a", [IN + 1, HID + 2 * HEADS[0]], BF,
                            kind="ExternalInput")
    gcn_in_w = nc.dram_tensor("gcn_in_w", [IN, HID], BF, kind="ExternalInput")
    gcn_res_w = nc.dram_tensor("gcn_res_w", [IN, HID], BF, kind="ExternalInput")
    gat_in_w = nc.dram_tensor("gat_in_w", [IN, HID], BF, kind="ExternalInput")
    gcn_w = nc.dram_tensor("gcn_w", [3, HID, HID], BF, kind="ExternalInput")
    gat_wg = [nc.dram_tensor(f"gat_wg{i}", [HID, HID + 2 * HEADS[i]], BF,
                             kind="ExternalInput") for i in range(3)]
    vecs = nc.dram_tensor("vecs", [128, 32], F32, kind="ExternalInput")
    gcn_c1w = nc.dram_tensor("gcn_c1w", [HID, 64], BF, kind="ExternalInput")
    gcn_c2w = nc.dram_tensor("gcn_c2w", [64, 32], BF, kind="ExternalInput")
    gcn_c3w = nc.dram_tensor("gcn_c3w", [32, HID], BF, kind="ExternalInput")
    gat_c1w = nc.dram_tensor("gat_c1w", [HID, 64], BF, kind="ExternalInput")
    gat_c2w = nc.dram_tensor("gat_c2w", [64, 32], BF, kind="ExternalInput")
    gat_c3w = nc.dram_tensor("gat_c3w", [32, HID], BF, kind="ExternalInput")
    fin_w = nc.dram_tensor("fin_w", [HID, 64], BF, kind="ExternalInput")
    fin2_w = nc.dram_tensor("fin2_w", [64, 2], BF, kind="ExternalInput")

    out_fm = nc.dram_tensor("out_fm", [2, BPC * BLK], F32, kind="ExternalOutput")

    VC = dict(gcn_in_b=0, gcn_res_b=1, gat_in_b=2,
              gcn_g=[3, 4, 5], gcn_bc=[6, 7, 8],
              gat_g=[9, 10, 11], gat_bc=[12, 13, 14],
              gcn_c1b=15, gcn_c2b=16, fused_b=17,
              gat_c1b=18, gat_c2b=19, fin_b=20, fin2_b=21, ones=22)

    with tile.TileContext(nc) as tc:
        with (
            tc.tile_pool(name="const", bufs=1) as cpool,
            tc.tile_pool(name="state", bufs=1) as spool,
            tc.tile_pool(name="work", bufs=3) as wpool,
            tc.tile_pool(name="gath", bufs=4) as gpool,
            tc.tile_pool(name="dltp", bufs=4) as dpool,
            tc.tile_pool(name="stage", bufs=2) as stpool,
            tc.tile_pool(name="psA", bufs=2, space="PSUM") as psA,      # agg accs
            tc.tile_pool(name="psE", bufs=2, space="PSUM") as psE,      # C expand
            tc.tile_pool(name="psB", bufs=1, space="PSUM") as psB,      # table mm
            tc.tile_pool(name="psF", bufs=1, space="PSUM") as psF,      # fusion
            tc.tile_pool(name="psC", bufs=2, space="PSUM") as psC,      # transients
            tc.tile_pool(name="dram", bufs=1, space="DRAM") as dram,
        ):
            nc.gpsimd.load_library(library_config.mlp)

            # ---- constants into SBUF ----
            _ldn = [0]

            def ld(shape, dt, src):
                _ldn[0] += 1
                t = cpool.tile(shape, dt, tag=f"c{_ldn[0]}")
                nc.sync.dma_start(t[:], src)
                return t

            idxA = ld([128, TILES * 8], I16, idx_a[:])
            idxB = ld([128, TILES * 8], I16, idx_b[:])
            dl = ld([128, TILA], BF, dstloc[:])
            iota = ld([128, TTA * 128], BF, iota_rep[:])
            iotac = ld([128, 1], F32, iotac_in[:])
            dis = ld([128, BPC], F32, dis_in[:])
            vec = ld([128, 32], F32, vecs[:])
            t0g = ld([IN + 1, HID], BF, t0g_in[:])
            t0a = ld([IN + 1, HID + 2 * HEADS[0]], BF, t0a_in[:])
            w_in = ld([IN, HID], BF, gcn_in_w[:])
            w_res = ld([IN, HID], BF, gcn_res_w[:])
            w_gin = ld([IN, HID], BF, gat_in_w[:])
            wg = [ld([HID, HID], BF, gcn_w[i, :, :]) for i in range(3)]
            wa = [ld([HID, HID + 2 * HEADS[i]], BF, gat_wg[i][:]) for i in range(3)]
            hw = {}
            for nm, hnd, shp in (
                ("gcn_c1w", gcn_c1w, [HID, 64]), ("gcn_c2w", gcn_c2w, [64, 32]),
                ("gcn_c3w", gcn_c3w, [32, HID]), ("gat_c1w", gat_c1w, [HID, 64]),
                ("gat_c2w", gat_c2w, [64, 32]), ("gat_c3w", gat_c3w, [32, HID]),
                ("fin_w", fin_w, [HID, 64]), ("fin2_w", fin2_w, [64, 2]),
            ):
                hw[nm] = ld(shp, BF, hnd[:])
            ident = cpool.tile([128, 128], BF, tag="ident")
            make_identity(nc, ident[:])
            x_sb = ld([IN + 1, BPC * BLK], BF, x_fm[:])

            def v(col, p=128):
                return vec[0:p, col:col + 1]

            # ---- persistent state (feature-major, bf16) ----
            xp = spool.tile([HID, BPC * BLK], BF, tag="xp")
            res = spool.tile([HID, BPC * BLK], BF, tag="res")
            xg = spool.tile([HID, BPC * BLK], BF, tag="xg")
            Cb = [spool.tile([128, BPC, 4], BF, tag=f"Cb{i}", name=f"Cb{i}")
                  for i in range(3)]
            # phase-A partial sums per block
            pacc = spool.tile([128, BPC, 260], BF, tag="pacc")

            # ---- DRAM table buffers ----
            agins = []   # [layer][half]
            tblsA = []
            tblsB = []
            for i in range(3):
                aa = dram.tile([HSH, TW], BF, name=f"agA{i}")
                ab = dram.tile([HSH, TW], BF, name=f"agB{i}")
                agins.append((aa, ab))
                tblsA.append(dram.tile([NC * HSH, TW], BF, name=f"tblA{i}"))
                tblsB.append(dram.tile([NC * HSH, TW], BF, name=f"tblB{i}"))

            def blk_sl(b):
                return slice(b * BLK, (b + 1) * BLK)

            _gq = [0]

            def emit_table(j, b, st, p1, p2):
                h = HEADS[j]
                nc.scalar.activation(st[:, 0:HID], p1[:], AF.Copy, bias=0.0,
                                     scale=dis[:, b:b + 1])
                nc.scalar.activation(st[:, HID:2 * HID], p2[:, 0:HID], AF.Copy,
                                     bias=0.0, scale=1.0)
                nc.scalar.activation(st[:, 256:256 + h], p2[:, HID:HID + h],
                                     AF.Exp, bias=0.0, scale=1.0)
                nc.scalar.activation(st[:, 256 + h:256 + 2 * h],
                                     p2[:, HID:HID + h],
                                     AF.Exp, bias=0.0, scale=0.2)
                nc.scalar.activation(Cb[j][:, b, 0:h], p2[:, HID + h:HID + 2 * h],
                                     AF.Exp, bias=0.0, scale=0.8)
                hb = b - BPC // 2 if b >= BPC // 2 else b
                nc.sync.dma_start(
                    agins[j][b >= BPC // 2][hb * BLK:(hb + 1) * BLK, :], st[:])

            def build_table_block(j, b):
                h = HEADS[j]
                st = stpool.tile([BLK, TW], BF, tag="tb")
                p1 = psB.tile([BLK, HID], F32, tag="tbl")
                nc.tensor.matmul(out=p1[:], lhsT=xp[:, blk_sl(b)], rhs=wg[j][:],
                                 start=True, stop=True)
                p2 = psB.tile([BLK, HID + 2 * h], F32, tag="tbl")
                nc.tensor.matmul(out=p2[:], lhsT=xg[:, blk_sl(b)], rhs=wa[j][:],
                                 start=True, stop=True)
                emit_table(j, b, st, p1, p2)

            def build_table0_block(b):
                h = HEADS[0]
                st = stpool.tile([BLK, TW], BF, tag="tb")
                p1 = psB.tile([BLK, HID], F32, tag="tbl")
                nc.tensor.matmul(out=p1[:], lhsT=x_sb[:, blk_sl(b)], rhs=t0g[:],
                                 start=True, stop=True)
                p2 = psB.tile([BLK, HID + 2 * h], F32, tag="tbl")
                nc.tensor.matmul(out=p2[:], lhsT=x_sb[:, blk_sl(b)], rhs=t0a[:],
                                 start=True, stop=True)
                emit_table(0, b, st, p1, p2)

            def ag_half(j, half):
                out_t = tblsB[j] if half else tblsA[j]
                nc.gpsimd.collective_compute(
                    "AllGather", ALU.bypass,
                    ins=[agins[j][half].opt()],
                    outs=[out_t[:]],
                    replica_groups=[list(range(NC))])

            def agg_phase(i, b, tblX, idxX, dl_off, self_tile, finish):
                """Gather T_half tiles (+optional self tile) and aggregate."""
                h = HEADS[i]
                C = HID // h
                W = 256 + h
                tt = T_half + (1 if self_tile else 0)
                t0 = b * T_half
                ta = b * TTT + dl_off
                _gq[0] += 1
                g = gpool.tile([128, TTA, TW], BF, tag="g")
                nc.gpsimd.dma_gather(
                    g[:, 0:T_half, :], tblX[:, 0:TW],
                    idxX[:, t0 * 8:(t0 + T_half) * 8],
                    T_half * 128, T_half * 128, TW, single_packet=False,
                    queue_num=_gq[0] % NQ)
                if self_tile:
                    hb = b - BPC // 2 if b >= BPC // 2 else b
                    nc.sync.dma_start(
                        g[:, T_half, :],
                        agins[i][b >= BPC // 2][hb * BLK:(hb + 1) * BLK, :])
                dt_ = dpool.tile([128, TTA * 128], BF, tag="dlt")
                nc.sync.dma_start(dt_[:, 0:tt * 128],
                                  dlt_in[:, ta * 128:(ta + tt) * 128])
                s = wpool.tile([128, TTA, 128], BF, tag="S")
                nc.vector.tensor_tensor(
                    out=s[:, 0:tt, :],
                    in0=iota[:, 0:tt * 128].rearrange("p (t k) -> p t k", k=128),
                    in1=dl[:, ta:ta + tt].to_broadcast([128, tt, 128]),
                    op=ALU.is_equal)
                s2 = wpool.tile([128, TTA * 128], BF, tag="S2")
                nc.vector.tensor_scalar(out=s2[:, 0:tt * 128],
                                        in0=dt_[:, 0:tt * 128],
                                        scalar1=iotac[:, 0:1], scalar2=None,
                                        op0=ALU.is_equal)
                o2 = psE.tile([128, TTA, 4], F32, tag="o2")
                for t in range(tt):
                    nc.tensor.matmul(
                        out=o2[:, t, 0:h], lhsT=s2[:, t * 128:(t + 1) * 128],
                        rhs=Cb[i][:, b, 0:h],
                        start=True, stop=True, skip_group_check=True)
                ce = wpool.tile([128, TTA, 4], BF, tag="ce")
                nc.scalar.activation(ce[:, 0:tt, 0:h], o2[:, 0:tt, 0:h],
                                     AF.Copy, bias=0.0, scale=1.0)
                e1 = wpool.tile([128, TTA, 4], BF, tag="e1")
                nc.vector.tensor_tensor(out=e1[:, 0:tt, 0:h],
                                        in0=g[:, 0:tt, 256:256 + h],
                                        in1=ce[:, 0:tt, 0:h], op=ALU.mult)
                nc.vector.tensor_tensor(out=g[:, 0:tt, 256:256 + h],
                                        in0=e1[:, 0:tt, 0:h],
                                        in1=g[:, 0:tt, 256 + h:256 + 2 * h],
                                        op=ALU.max)
                nc.vector.tensor_tensor(
                    out=g[:, 0:tt, HID:2 * HID].rearrange(
                        "p t (h c) -> p t h c", c=C),
                    in0=g[:, 0:tt, HID:2 * HID].rearrange(
                        "p t (h c) -> p t h c", c=C),
                    in1=g[:, 0:tt, 256:256 + h].to_broadcast(
                        [128, tt, h, C]),
                    op=ALU.mult)
                acc = psA.tile([128, 260], F32, tag="acc")
                for t in range(tt):
                    nc.tensor.matmul(
                        out=acc[:, 0:W], lhsT=s[:, t, :], rhs=g[:, t, 0:W],
                        start=(t == 0), stop=(t == tt - 1),
                        skip_group_check=True)
                finish(acc)

            def do_heads(b):
                p1 = psC.tile([64, BLK], F32, tag="tmp")
                nc.tensor.matmul(out=p1[:], lhsT=hw["gcn_c1w"][:],
                                 rhs=xp[:, blk_sl(b)], start=True, stop=True)
                a1 = stpool.tile([64, BLK], BF, tag="a1")
                nc.scalar.activation(a1[:], p1[:], AF.Relu,
                                     bias=v(VC["gcn_c1b"], 64), scale=1.0)
                p2 = psC.tile([32, BLK], F32, tag="tmp")
                nc.tensor.matmul(out=p2[:], lhsT=hw["gcn_c2w"][:], rhs=a1[:],
                                 start=True, stop=True)
                a2 = stpool.tile([32, BLK], BF, tag="a2")
                nc.scalar.activation(a2[:], p2[:], AF.Relu,
                                     bias=v(VC["gcn_c2b"], 32), scale=1.0)
                pf = psF.tile([HID, BLK], F32, tag="fuse")
                nc.tensor.matmul(out=pf[:], lhsT=hw["gcn_c3w"][:], rhs=a2[:],
                                 start=True, stop=False, skip_group_check=True)
                p3 = psC.tile([64, BLK], F32, tag="tmp")
                nc.tensor.matmul(out=p3[:], lhsT=hw["gat_c1w"][:],
                                 rhs=xg[:, blk_sl(b)], start=True, stop=True)
                b1 = stpool.tile([64, BLK], BF, tag="a1")
                nc.scalar.activation(b1[:], p3[:], AF.Relu,
                                     bias=v(VC["gat_c1b"], 64), scale=1.0)
                p4 = psC.tile([32, BLK], F32, tag="tmp")
                nc.tensor.matmul(out=p4[:], lhsT=hw["gat_c2w"][:], rhs=b1[:],
                                 start=True, stop=True)
                b2 = stpool.tile([32, BLK], BF, tag="a2")
                nc.scalar.activation(b2[:], p4[:], AF.Relu,
                                     bias=v(VC["gat_c2b"], 32), scale=1.0)
                nc.tensor.matmul(out=pf[:], lhsT=hw["gat_c3w"][:], rhs=b2[:],
                                 start=False, stop=True, skip_group_check=True)
                fs = stpool.tile([HID, BLK], BF, tag="fs")
                nc.scalar.activation(fs[:], pf[:], AF.Identity,
                                     bias=v(VC["fused_b"]), scale=1.0)
                p5 = psC.tile([64, BLK], F32, tag="tmp")
                nc.tensor.matmul(out=p5[:], lhsT=hw["fin_w"][:], rhs=fs[:],
                                 start=True, stop=True)
                f1 = stpool.tile([64, BLK], BF, tag="a1")
                nc.scalar.activation(f1[:], p5[:], AF.Relu,
                                     bias=v(VC["fin_b"], 64), scale=1.0)
                p6 = psC.tile([2, BLK], F32, tag="tmp")
                nc.tensor.matmul(out=p6[:], lhsT=hw["fin2_w"][:], rhs=f1[:],
                                 start=True, stop=True)
                oo = stpool.tile([2, BLK], F32, tag="oo")
                nc.scalar.activation(oo[:], p6[:], AF.Identity,
                                     bias=v(VC["fin2_b"], 2), scale=1.0)
                nc.sync.dma_start(out_fm[:, blk_sl(b)], oo[:])

            # ---- prologue: table 0 from x (AGs asap), then states ----
            for b in range(BPC):
                build_table0_block(b)
                if b == BPC // 2 - 1:
                    ag_half(0, 0)
            ag_half(0, 1)
            for b in range(BPC):
                for (w_sb, bias_col, dst_state) in (
                    (w_in, VC["gcn_in_b"], xp),
                    (w_res, VC["gcn_res_b"], res),
                    (w_gin, VC["gat_in_b"], xg),
                ):
                    p = psC.tile([HID, BLK], F32, tag="tmp")
                    nc.tensor.matmul(out=p[:], lhsT=w_sb[:],
                                     rhs=x_sb[0:IN, blk_sl(b)],
                                     start=True, stop=True)
                    nc.scalar.activation(dst_state[:, blk_sl(b)], p[:], AF.Identity,
                                         bias=v(bias_col), scale=1.0)

            # ================= merged round: GCN + GAT layer i =================
            def round_layer(i):
                h = HEADS[i]
                C = HID // h
                W = 256 + h
                gcol, bcol = VC["gcn_g"][i], VC["gcn_bc"][i]
                gcol2, bcol2 = VC["gat_g"][i], VC["gat_bc"][i]

                # ---- phase A: half-0 sources + self tile -> partial sums ----
                for b in range(BPC):
                    def finishA(acc, b=b):
                        nc.scalar.activation(pacc[:, b, 0:W], acc[:, 0:W],
                                             AF.Copy, bias=0.0, scale=1.0)
                    agg_phase(i, b, tblsA[i], idxA, 0, True, finishA)

                # ---- phase B: half-1 sources, finish, epilogues ----
                for b in range(BPC):
                    def finishB(acc, b=b):
                        fs = stpool.tile([128, 260], F32, tag="facc")
                        nc.vector.tensor_tensor(out=fs[:, 0:W],
                                                in0=acc[:, 0:W],
                                                in1=pacc[:, b, 0:W],
                                                op=ALU.add)
                        # GCN epilogue
                        u = stpool.tile([BLK, HID], BF, tag="gcn_u")
                        nc.scalar.activation(u[:], fs[:, 0:HID], AF.Copy,
                                             bias=0.0, scale=dis[:, b:b + 1])
                        tp = psC.tile([HID, BLK], BF, tag="tmp")
                        nc.tensor.transpose(tp[:], u[:], ident[:])
                        if i == 1:
                            xn = stpool.tile([HID, BLK], BF, tag="gcn_xn")
                            nc.scalar.activation(xn[:], tp[:], AF.Relu,
                                                 bias=v(bcol), scale=v(gcol))
                            nc.vector.tensor_tensor(out=xp[:, blk_sl(b)],
                                                    in0=xn[:],
                                                    in1=res[:, blk_sl(b)],
                                                    op=ALU.add)
                        else:
                            nc.scalar.activation(xp[:, blk_sl(b)], tp[:],
                                                 AF.Relu,
                                                 bias=v(bcol), scale=v(gcol))
                        # GAT epilogue
                        rr = stpool.tile([BLK, 4], F32, tag="rr")
                        nc.vector.tensor_scalar(out=rr[0:BLK, 0:h],
                                                in0=fs[:, 256:256 + h],
                                                scalar1=1e-16, scalar2=None,
                                                op0=ALU.add)
                        nc.vector.reciprocal(rr[0:BLK, 0:h], rr[0:BLK, 0:h])
                        o = stpool.tile([BLK, HID], BF, tag="gat_o")
                        for hh in range(h):
                            nc.scalar.activation(
                                o[:, hh * C:(hh + 1) * C],
                                fs[:, HID + hh * C:HID + (hh + 1) * C],
                                AF.Copy, bias=0.0,
                                scale=rr[0:BLK, hh:hh + 1])
                        tp2 = psC.tile([HID, BLK], BF, tag="tmp")
                        nc.tensor.transpose(tp2[:], o[:], ident[:])
                        # elu(u) = relu(u) - relu(1 - exp(u))
                        r1 = stpool.tile([HID, BLK], BF, tag="gat_r1")
                        q = stpool.tile([HID, BLK], BF, tag="gat_q")
                        r2 = stpool.tile([HID, BLK], BF, tag="gat_r2")
                        nc.scalar.activation(r1[:], tp2[:], AF.Relu,
                                             bias=v(bcol2), scale=v(gcol2))
                        nc.scalar.activation(q[:], tp2[:], AF.Exp,
                                             bias=v(bcol2), scale=v(gcol2))
                        nc.scalar.activation(r2[:], q[:], AF.Relu,
                                             bias=v(VC["ones"]), scale=-1.0)
                        nc.vector.tensor_tensor(out=xg[:, blk_sl(b)],
                                                in0=r1[:], in1=r2[:],
                                                op=ALU.subtract)
                        # next round's table, or final heads
                        if i + 1 < 3:
                            build_table_block(i + 1, b)
                            if b == BPC // 2 - 1:
                                ag_half(i + 1, 0)
                        else:
                            do_heads(b)
                    agg_phase(i, b, tblsB[i], idxB, TTA, False, finishB)
                if i + 1 < 3:
                    ag_half(i + 1, 1)

            for i in range(min(ngcn, 3)):
                round_layer(i)

    nc.compile()
    return nc


# ----------------------------------------------------------------------------
# Host orchestration
# ----------------------------------------------------------------------------

def make_inputs_per_core(inputs, src_pads, dst_pad, dstloc, dis, T_half):
    srcA, srcB = src_pads
    TTT = 2 * T_half + 1
    TILA = BPC * TTT

    x = np.asarray(inputs["x"], np.float32)
    xpad = np.zeros((NPAD, IN + 1), np.float32)
    xpad[:N, :IN] = x
    xpad[:, IN] = 1.0

    g = lambda k: np.asarray(inputs[k], np.float32)

    vecs = np.zeros((128, 32), np.float32)

    def setv(col, arr):
        arr = np.asarray(arr, np.float32).ravel()
        vecs[: arr.shape[0], col] = arr

    VC = dict(gcn_in_b=0, gcn_res_b=1, gat_in_b=2,
              gcn_g=[3, 4, 5], gcn_bc=[6, 7, 8],
              gat_g=[9, 10, 11], gat_bc=[12, 13, 14],
              gcn_c1b=15, gcn_c2b=16, fused_b=17,
              gat_c1b=18, gat_c2b=19, fin_b=20, fin2_b=21, ones=22)
    setv(VC["gcn_in_b"], g("gcn_in_b"))
    setv(VC["gcn_res_b"], g("gcn_res_b"))
    setv(VC["gat_in_b"], g("gat_in_b"))
    for i in range(3):
        gp = bn_fold(g("gcn_bn_g")[i])
        setv(VC["gcn_g"][i], gp)
        setv(VC["gcn_bc"][i], g("gcn_b")[i] * gp + g("gcn_bn_b")[i])
        gp2 = bn_fold(g("gat_bn_g")[i])
        setv(VC["gat_g"][i], gp2)
        setv(VC["gat_bc"][i], g("gat_b")[i] * gp2 + g("gat_bn_b")[i])
    setv(VC["gcn_c1b"], g("gcn_c1b"))
    setv(VC["gcn_c2b"], g("gcn_c2b"))
    setv(VC["fused_b"], 0.6 * g("gcn_c3b") + 0.4 * g("gat_c3b"))
    setv(VC["gat_c1b"], g("gat_c1b"))
    cg = bn_fold(g("gat_cbn_g"))
    setv(VC["gat_c2b"], g("gat_cbn_b") @ g("gat_c2w") + g("gat_c2b"))
    setv(VC["fin_b"], g("fin_b"))
    fg = bn_fold(g("fin_bn_g"))
    setv(VC["fin2_b"], g("fin_bn_b") @ g("fin2_w") + g("fin2_b"))
    setv(VC["ones"], np.ones(128, np.float32))

    gcn_c3wf = 0.6 * g("gcn_c3w")
    gat_c1wf = g("feat_imp")[:, None] * g("gat_c1w")
    gat_c2wf = cg[:, None] * g("gat_c2w")
    gat_c3wf = 0.4 * g("gat_c3w")
    fin2_wf = fg[:, None] * g("fin2_w")

    gat_wgs = []
    for i in range(3):
        W = g("gat_w")[i]
        asrc_bd = make_blockdiag(g("gat_asrc")[i], HEADS[i])
        adst_bd = make_blockdiag(g("gat_adst")[i], HEADS[i])
        gat_wgs.append(np.concatenate([W, W @ asrc_bd, W @ adst_bd], axis=1)
                       .astype(np.float32))

    t0g = np.vstack([g("gcn_in_w") @ g("gcn_w")[0],
                     g("gcn_in_b") @ g("gcn_w")[0]])
    t0a = np.vstack([g("gat_in_w") @ gat_wgs[0],
                     g("gat_in_b") @ gat_wgs[0]])

    iota = np.tile(np.arange(128, dtype=np.float32)[None, :],
               (128, T_half + 1)).astype(NPBF)
    iotac = np.arange(128, dtype=np.float32).reshape(128, 1)

    in_maps = []
    for c in range(NC):
        b0 = c * BPC
        nodes = slice(b0 * BLK, (b0 + BPC) * BLK)
        sgA = srcA[b0:b0 + BPC].reshape(-1, 128)      # relative tblA rows
        sgB = srcB[b0:b0 + BPC].reshape(-1, 128)
        dloc = dstloc[b0:b0 + BPC].reshape(-1, 128)   # [TILA, 128] int32
        im = dict(
            x_fm=np.ascontiguousarray(xpad[nodes].T).astype(NPBF),
            idx_a=wrap_idx_core(sgA),
            idx_b=wrap_idx_core(sgB),
            dstloc=np.ascontiguousarray(dloc.T.astype(np.float32)).astype(NPBF),
            dlt=np.broadcast_to(dloc.reshape(1, -1).astype(np.float32)
                    .astype(NPBF), (128, TILA * 128)).copy(),
            iota_rep=iota,
            iotac=iotac,
            dis=np.ascontiguousarray(dis[nodes].reshape(BPC, BLK).T),
            t0g=t0g.astype(NPBF), t0a=t0a.astype(NPBF),
            gcn_in_w=g("gcn_in_w").astype(NPBF),
            gcn_res_w=g("gcn_res_w").astype(NPBF),
            gat_in_w=g("gat_in_w").astype(NPBF),
            gcn_w=g("gcn_w").astype(NPBF),
            gat_wg0=gat_wgs[0].astype(NPBF), gat_wg1=gat_wgs[1].astype(NPBF),
            gat_wg2=gat_wgs[2].astype(NPBF),
            vecs=vecs,
            gcn_c1w=g("gcn_c1w").astype(NPBF),
            gcn_c2w=g("gcn_c2w").astype(NPBF),
            gcn_c3w=gcn_c3wf.astype(NPBF),
            gat_c1w=gat_c1wf.astype(NPBF),
            gat_c2w=gat_c2wf.astype(NPBF),
            gat_c3w=gat_c3wf.astype(NPBF),
            fin_w=g("fin_w").astype(NPBF),
            fin2_w=fin2_wf.astype(NPBF),
        )
        in_maps.append(im)
    return in_maps


_CACHE = {}


def _get_program(T_half):
    if T_half not in _CACHE:
        _CACHE[T_half] = build_program(T_half)
    return _CACHE[T_half]


def kernel(**inputs):
    edge_index = np.asarray(inputs["edge_index"])
    src_pads, dst_pad, dstloc, dis, T_half = preprocess_graph(edge_index)
    in_maps = make_inputs_per_core(inputs, src_pads, dst_pad, dstloc, dis,
                                   T_half)
    nc = _get_program(T_half)
    res = run_bass_kernel_spmd(nc, in_maps, core_ids=list(range(NC)))
    out = np.concatenate([res.results[c]["out_fm"].T for c in range(NC)], axis=0)
    return np.ascontiguousarray(out[:N]).astype(np.float32)


# revision 19
# speedup vs baseline: 1.1901x; 1.1901x over previous
"""Trainium2 Bass kernel for nn_AdvancedHybridGNN (hybrid GCN+GAT, N=30000, E=600000).

Strategy (8 NeuronCores, graph/data parallel; V6):
- Nodes padded to 30720 = 8 cores x 30 blocks x 128. Real edges sorted by
  destination block and SPLIT BY SOURCE HALF (global table rows for blocks 0-14
  of every core = tblA, blocks 15-29 = tblB), each half padded per block to
  T_half tiles of 128. Self-loops are one extra "self tile" per block loaded
  from the local pre-AllGather buffer with dma_start (no gather descriptors).
- Per layer the table (row = [gcn(128) | xh(128) | A=exp(asrc)(h) |
  A'=exp(.2 asrc)(h) | pad], 384 bf16 cols = 768B) is AllGathered into TWO
  per-half Shared DRAM tensors, each written by exactly one collective. A round
  runs in two phases:
    phase A: per block gather half-0 source rows (+ self tile), weight, and
      aggregate via one-hot S matmuls; evict partial sums to SBUF (bf16).
      Only needs tblA (AllGather fired mid-previous-round), so it overlaps the
      in-flight half-1 AllGather.
    phase B: gather half-1 rows, aggregate, add phase-A partials, epilogues,
      build next layer's table rows, fire the next AllGathers (half-0
      mid-phase, half-1 at the end). Round 2 runs the output heads here.
- GAT softmax factored exactly: dividing num and denom by exp(.2 adst) gives
  per-edge weight ex = max(A * C_dst, A') with C = exp(.8 adst) a per-dst-block
  [128, h] local tile, expanded per edge with a tiny matmul lhsT=S2 (dst-major
  one-hot built on DVE from int8 replicated dstloc rows).
- Layer-0 table built directly from x via host-fused [x;1] weights.
- GCN sym-norm folded as dis[src] into the table and dis[dst] into the
  epilogue; eval BatchNorms folded into per-feature affines; bf16 on SBUF.
"""
import numpy as np
import sys

sys.path.insert(0, "/opt/trn_rl_repo")

import concourse.bacc as bacc
import concourse.bass as bass
import concourse.mybir as mybir
import concourse.tile as tile
from concourse import library_config
from concourse.masks import make_identity
from concourse.bass_utils import run_bass_kernel_spmd

F32 = mybir.dt.float32
BF = mybir.dt.bfloat16
I16 = mybir.dt.int16
I8 = mybir.dt.int8
NPBF = mybir.dt.np(BF)
AF = mybir.ActivationFunctionType
ALU = mybir.AluOpType

N = 30000
NPAD = 30720
E = 600000
IN = 64
HID = 128
EPS = 1e-5
HEADS = (4, 4, 1)
NC = 8
BLK = 128
NBLK = NPAD // BLK      # 240
BPC = NBLK // NC        # 30 blocks per core
TW = 384                # table row width (768 B in bf16; must be %256B)


# ----------------------------------------------------------------------------
# Host-side graph preprocessing
# ----------------------------------------------------------------------------

def preprocess_graph(edge_index):
    src = edge_index[0].astype(np.int64)
    dst = edge_index[1].astype(np.int64)
    # degree includes the self loop (reference adds them)
    deg = np.bincount(dst, minlength=NPAD) + 1
    deg[N:] = 0
    dis = np.zeros(NPAD, np.float32)
    m = deg > 0
    dis[m] = (1.0 / np.sqrt(deg[m].astype(np.float64))).astype(np.float32)

    # permuted-AllGather row id: blocks 0-14 of each core -> tblA, 15-29 -> tblB
    c2 = src // (BPC * BLK)
    k2 = src % (BPC * BLK)
    sh = BPC * BLK // 2
    in_h1 = k2 >= sh
    row = np.where(in_h1, c2 * sh + (k2 - sh), c2 * sh + k2)  # half-relative

    key = dst * 2 + in_h1
    order = np.argsort(key, kind="stable")
    src_r = row[order]
    dst_s = dst[order]
    h1_s = in_h1[order]
    key_s = key[order]
    blk_of = dst_s // BLK
    c0 = np.bincount(blk_of[~h1_s], minlength=NBLK)
    c1 = np.bincount(blk_of[h1_s], minlength=NBLK)
    T_half = max(1, int(np.ceil(max(c0.max(), c1.max()) / 128)))
    EPH = T_half * BLK
    TTT = 2 * T_half + 1
    srcA = np.zeros((NBLK, EPH), np.int64)
    srcB = np.zeros((NBLK, EPH), np.int64)
    # per block tile layout: [h0 (T_half) | self (1) | h1 (T_half)]
    dstloc = np.full((NBLK, TTT * BLK), -1, np.int32)
    dstloc[:, EPH:EPH + BLK] = np.arange(BLK, dtype=np.int32)[None, :]
    for b in range(NBLK):
        lo = np.searchsorted(key_s, (b * BLK) * 2)
        hi = np.searchsorted(key_s, ((b + 1) * BLK) * 2)
        seg_dst = dst_s[lo:hi]
        seg_h1 = h1_s[lo:hi]
        seg_src = src_r[lo:hi]
        m0 = ~seg_h1
        n0 = int(m0.sum())
        n1 = int(len(seg_dst) - n0)
        srcA[b, :n0] = seg_src[m0]
        srcB[b, :n1] = seg_src[seg_h1]
        dstloc[b, :n0] = (seg_dst[m0] - b * BLK).astype(np.int32)
        dstloc[b, EPH + BLK:EPH + BLK + n1] = \
            (seg_dst[seg_h1] - b * BLK).astype(np.int32)
    return (srcA, srcB), None, dstloc, dis, T_half


def wrap_idx_core(idx_tiles):
    """idx_tiles: [T, 128] int -> int16 gather layout [128, T*8]."""
    T = idx_tiles.shape[0]
    a = idx_tiles.reshape(T, 8, 16).transpose(2, 0, 1).reshape(16, T * 8)
    return np.tile(a, (8, 1)).astype(np.int16)


def bn_fold(g):
    return g / np.sqrt(1.0 + EPS)


def make_blockdiag(a, heads):
    C = HID // heads
    bd = np.zeros((HID, heads), np.float32)
    for h in range(heads):
        bd[h * C:(h + 1) * C, h] = a[h * C:(h + 1) * C]
    return bd


# ----------------------------------------------------------------------------
# Device program
# ----------------------------------------------------------------------------

NQ = 4  # SWDGE queues for gather parallelism


def build_program(T_half, ngcn=3):
    """Build the SPMD Bass program (same for all 8 cores)."""
    nc = bacc.Bacc("TRN2", num_swdge_queues=NQ)
    TTA = T_half + 1                    # phase-A tiles (h0 + self)
    TTT = 2 * T_half + 1
    TILES = BPC * T_half                # gathered tiles per half per core
    TILA = BPC * TTT                    # all tiles per core
    HSH = BPC * BLK // 2                # 1920 rows per half-shard

    # ---- inputs ----
    x_fm = nc.dram_tensor("x_fm", [IN + 1, BPC * BLK], BF, kind="ExternalInput")
    idx_a = nc.dram_tensor("idx_a", [128, TILES * 8], I16, kind="ExternalInput")
    idx_b = nc.dram_tensor("idx_b", [128, TILES * 8], I16, kind="ExternalInput")
    dstloc = nc.dram_tensor("dstloc", [128, TILA], I8, kind="ExternalInput")
    dlt_in = nc.dram_tensor("dlt", [128, TILA * 128], I8, kind="ExternalInput")
    iota_rep = nc.dram_tensor("iota_rep", [128, TTA * 128], I8, kind="ExternalInput")
    iotac_in = nc.dram_tensor("iotac", [128, 1], F32, kind="ExternalInput")
    dis_in = nc.dram_tensor("dis", [128, BPC], F32, kind="ExternalInput")

    t0g_in = nc.dram_tensor("t0g", [IN + 1, HID], BF, kind="ExternalInput")
    t0a_in = nc.dram_tensor("t0a", [IN + 1, HID + 2 * HEADS[0]], BF,
                            kind="ExternalInput")
    gcn_in_w = nc.dram_tensor("gcn_in_w", [IN, HID], BF, kind="ExternalInput")
    gcn_res_w = nc.dram_tensor("gcn_res_w", [IN, HID], BF, kind="ExternalInput")
    gat_in_w = nc.dram_tensor("gat_in_w", [IN, HID], BF, kind="ExternalInput")
    gcn_w = nc.dram_tensor("gcn_w", [3, HID, HID], BF, kind="ExternalInput")
    gat_wg = [nc.dram_tensor(f"gat_wg{i}", [HID, HID + 2 * HEADS[i]], BF,
                             kind="ExternalInput") for i in range(3)]
    vecs = nc.dram_tensor("vecs", [128, 32], F32, kind="ExternalInput")
    gcn_c1w = nc.dram_tensor("gcn_c1w", [HID, 64], BF, kind="ExternalInput")
    gcn_c2w = nc.dram_tensor("gcn_c2w", [64, 32], BF, kind="ExternalInput")
    gcn_c3w = nc.dram_tensor("gcn_c3w", [32, HID], BF, kind="ExternalInput")
    gat_c1w = nc.dram_tensor("gat_c1w", [HID, 64], BF, kind="ExternalInput")
    gat_c2w = nc.dram_tensor("gat_c2w", [64, 32], BF, kind="ExternalInput")
    gat_c3w = nc.dram_tensor("gat_c3w", [32, HID], BF, kind="ExternalInput")
    fin_w = nc.dram_tensor("fin_w", [HID, 64], BF, kind="ExternalInput")
    fin2_w = nc.dram_tensor("fin2_w", [64, 2], BF, kind="ExternalInput")

    out_fm = nc.dram_tensor("out_fm", [2, BPC * BLK], F32, kind="ExternalOutput")

    VC = dict(gcn_in_b=0, gcn_res_b=1, gat_in_b=2,
              gcn_g=[3, 4, 5], gcn_bc=[6, 7, 8],
              gat_g=[9, 10, 11], gat_bc=[12, 13, 14],
              gcn_c1b=15, gcn_c2b=16, fused_b=17,
              gat_c1b=18, gat_c2b=19, fin_b=20, fin2_b=21, ones=22)

    with tile.TileContext(nc) as tc:
        with (
            tc.tile_pool(name="const", bufs=1) as cpool,
            tc.tile_pool(name="state", bufs=1) as spool,
            tc.tile_pool(name="work", bufs=4) as wpool,
            tc.tile_pool(name="gath", bufs=5) as gpool,
            tc.tile_pool(name="dltp", bufs=8) as dpool,
            tc.tile_pool(name="stage", bufs=2) as stpool,
            tc.tile_pool(name="psA", bufs=2, space="PSUM") as psA,      # agg accs
            tc.tile_pool(name="psE", bufs=2, space="PSUM") as psE,      # C expand
            tc.tile_pool(name="psB", bufs=1, space="PSUM") as psB,      # table mm
            tc.tile_pool(name="psF", bufs=1, space="PSUM") as psF,      # fusion
            tc.tile_pool(name="psC", bufs=2, space="PSUM") as psC,      # transients
            tc.tile_pool(name="dram", bufs=1, space="DRAM") as dram,
        ):
            nc.gpsimd.load_library(library_config.mlp)

            # ---- constants into SBUF ----
            _ldn = [0]

            def ld(shape, dt, src):
                _ldn[0] += 1
                t = cpool.tile(shape, dt, tag=f"c{_ldn[0]}")
                nc.sync.dma_start(t[:], src)
                return t

            idxA = ld([128, TILES * 8], I16, idx_a[:])
            idxB = ld([128, TILES * 8], I16, idx_b[:])
            dl = ld([128, TILA], I8, dstloc[:])
            iota = ld([128, TTA * 128], I8, iota_rep[:])
            iotac = ld([128, 1], F32, iotac_in[:])
            dis = ld([128, BPC], F32, dis_in[:])
            vec = ld([128, 32], F32, vecs[:])
            t0g = ld([IN + 1, HID], BF, t0g_in[:])
            t0a = ld([IN + 1, HID + 2 * HEADS[0]], BF, t0a_in[:])
            w_in = ld([IN, HID], BF, gcn_in_w[:])
            w_res = ld([IN, HID], BF, gcn_res_w[:])
            w_gin = ld([IN, HID], BF, gat_in_w[:])
            wg = [ld([HID, HID], BF, gcn_w[i, :, :]) for i in range(3)]
            wa = [ld([HID, HID + 2 * HEADS[i]], BF, gat_wg[i][:]) for i in range(3)]
            hw = {}
            for nm, hnd, shp in (
                ("gcn_c1w", gcn_c1w, [HID, 64]), ("gcn_c2w", gcn_c2w, [64, 32]),
                ("gcn_c3w", gcn_c3w, [32, HID]), ("gat_c1w", gat_c1w, [HID, 64]),
                ("gat_c2w", gat_c2w, [64, 32]), ("gat_c3w", gat_c3w, [32, HID]),
                ("fin_w", fin_w, [HID, 64]), ("fin2_w", fin2_w, [64, 2]),
            ):
                hw[nm] = ld(shp, BF, hnd[:])
            ident = cpool.tile([128, 128], BF, tag="ident")
            make_identity(nc, ident[:])
            x_sb = ld([IN + 1, BPC * BLK], BF, x_fm[:])

            def v(col, p=128):
                return vec[0:p, col:col + 1]

            # ---- persistent state (feature-major, bf16) ----
            xp = spool.tile([HID, BPC * BLK], BF, tag="xp")
            res = spool.tile([HID, BPC * BLK], BF, tag="res")
            xg = spool.tile([HID, BPC * BLK], BF, tag="xg")
            Cb = [spool.tile([128, BPC, 4], BF, tag=f"Cb{i}", name=f"Cb{i}")
                  for i in range(3)]
            # phase-A partial sums per block
            pacc = spool.tile([128, BPC, 260], BF, tag="pacc")

            # ---- DRAM table buffers ----
            agins = []   # [layer][half]
            tblsA = []
            tblsB = []
            for i in range(3):
                aa = dram.tile([HSH, TW], BF, name=f"agA{i}")
                ab = dram.tile([HSH, TW], BF, name=f"agB{i}")
                agins.append((aa, ab))
                tblsA.append(dram.tile([NC * HSH, TW], BF, name=f"tblA{i}",
                                       addr_space="Shared"))
                tblsB.append(dram.tile([NC * HSH, TW], BF, name=f"tblB{i}",
                                       addr_space="Shared"))

            def blk_sl(b):
                return slice(b * BLK, (b + 1) * BLK)

            _gq = [0]

            def emit_table(j, b, st, p1, p2):
                h = HEADS[j]
                nc.scalar.activation(st[:, 0:HID], p1[:], AF.Copy, bias=0.0,
                                     scale=dis[:, b:b + 1])
                nc.scalar.activation(st[:, HID:2 * HID], p2[:, 0:HID], AF.Copy,
                                     bias=0.0, scale=1.0)
                nc.scalar.activation(st[:, 256:256 + h], p2[:, HID:HID + h],
                                     AF.Exp, bias=0.0, scale=1.0)
                nc.scalar.activation(st[:, 256 + h:256 + 2 * h],
                                     p2[:, HID:HID + h],
                                     AF.Exp, bias=0.0, scale=0.2)
                nc.scalar.activation(Cb[j][:, b, 0:h], p2[:, HID + h:HID + 2 * h],
                                     AF.Exp, bias=0.0, scale=0.8)
                hb = b - BPC // 2 if b >= BPC // 2 else b
                nc.sync.dma_start(
                    agins[j][b >= BPC // 2][hb * BLK:(hb + 1) * BLK, :], st[:])

            def build_table_block(j, b):
                h = HEADS[j]
                st = stpool.tile([BLK, TW], BF, tag="tb")
                p1 = psB.tile([BLK, HID], F32, tag="tbl")
                nc.tensor.matmul(out=p1[:], lhsT=xp[:, blk_sl(b)], rhs=wg[j][:],
                                 start=True, stop=True)
                p2 = psB.tile([BLK, HID + 2 * h], F32, tag="tbl")
                nc.tensor.matmul(out=p2[:], lhsT=xg[:, blk_sl(b)], rhs=wa[j][:],
                                 start=True, stop=True)
                emit_table(j, b, st, p1, p2)

            def build_table0_block(b):
                h = HEADS[0]
                st = stpool.tile([BLK, TW], BF, tag="tb")
                p1 = psB.tile([BLK, HID], F32, tag="tbl")
                nc.tensor.matmul(out=p1[:], lhsT=x_sb[:, blk_sl(b)], rhs=t0g[:],
                                 start=True, stop=True)
                p2 = psB.tile([BLK, HID + 2 * h], F32, tag="tbl")
                nc.tensor.matmul(out=p2[:], lhsT=x_sb[:, blk_sl(b)], rhs=t0a[:],
                                 start=True, stop=True)
                emit_table(0, b, st, p1, p2)

            def ag_half(j, half):
                out_t = tblsB[j] if half else tblsA[j]
                nc.gpsimd.collective_compute(
                    "AllGather", ALU.bypass,
                    ins=[agins[j][half].opt()],
                    outs=[out_t[:]],
                    replica_groups=[list(range(NC))])

            def agg_phase(i, b, tblX, idxX, dl_off, self_tile, finish):
                """Gather T_half tiles (+optional self tile) and aggregate."""
                h = HEADS[i]
                C = HID // h
                W = 256 + h
                tt = T_half + (1 if self_tile else 0)
                t0 = b * T_half
                ta = b * TTT + dl_off
                _gq[0] += 1
                g = gpool.tile([128, TTA, TW], BF, tag="g")
                nc.gpsimd.dma_gather(
                    g[:, 0:T_half, :], tblX[:, 0:TW],
                    idxX[:, t0 * 8:(t0 + T_half) * 8],
                    T_half * 128, T_half * 128, TW, single_packet=False,
                    queue_num=_gq[0] % NQ)
                if self_tile:
                    hb = b - BPC // 2 if b >= BPC // 2 else b
                    nc.sync.dma_start(
                        g[:, T_half, :],
                        agins[i][b >= BPC // 2][hb * BLK:(hb + 1) * BLK, :])
                dt_ = dpool.tile([128, TTA * 128], I8, tag="dlt")
                nc.sync.dma_start(dt_[:, 0:tt * 128],
                                  dlt_in[:, ta * 128:(ta + tt) * 128])
                s = wpool.tile([128, TTA, 128], BF, tag="S")
                nc.vector.tensor_tensor(
                    out=s[:, 0:tt, :],
                    in0=iota[:, 0:tt * 128].rearrange("p (t k) -> p t k", k=128),
                    in1=dl[:, ta:ta + tt].to_broadcast([128, tt, 128]),
                    op=ALU.is_equal)
                s2 = wpool.tile([128, TTA * 128], BF, tag="S2")
                nc.vector.tensor_scalar(out=s2[:, 0:tt * 128],
                                        in0=dt_[:, 0:tt * 128],
                                        scalar1=iotac[:, 0:1], scalar2=None,
                                        op0=ALU.is_equal)
                o2 = psE.tile([128, TTA, 4], F32, tag="o2")
                for t in range(tt):
                    nc.tensor.matmul(
                        out=o2[:, t, 0:h], lhsT=s2[:, t * 128:(t + 1) * 128],
                        rhs=Cb[i][:, b, 0:h],
                        start=True, stop=True, skip_group_check=True)
                ce = wpool.tile([128, TTA, 4], BF, tag="ce")
                nc.scalar.activation(ce[:, 0:tt, 0:h], o2[:, 0:tt, 0:h],
                                     AF.Copy, bias=0.0, scale=1.0)
                e1 = wpool.tile([128, TTA, 4], BF, tag="e1")
                nc.vector.tensor_tensor(out=e1[:, 0:tt, 0:h],
                                        in0=g[:, 0:tt, 256:256 + h],
                                        in1=ce[:, 0:tt, 0:h], op=ALU.mult)
                nc.vector.tensor_tensor(out=g[:, 0:tt, 256:256 + h],
                                        in0=e1[:, 0:tt, 0:h],
                                        in1=g[:, 0:tt, 256 + h:256 + 2 * h],
                                        op=ALU.max)
                nc.vector.tensor_tensor(
                    out=g[:, 0:tt, HID:2 * HID].rearrange(
                        "p t (h c) -> p t h c", c=C),
                    in0=g[:, 0:tt, HID:2 * HID].rearrange(
                        "p t (h c) -> p t h c", c=C),
                    in1=g[:, 0:tt, 256:256 + h].to_broadcast(
                        [128, tt, h, C]),
                    op=ALU.mult)
                acc = psA.tile([128, 260], F32, tag="acc")
                for t in range(tt):
                    nc.tensor.matmul(
                        out=acc[:, 0:W], lhsT=s[:, t, :], rhs=g[:, t, 0:W],
                        start=(t == 0), stop=(t == tt - 1),
                        skip_group_check=True)
                finish(acc)

            def do_heads(b):
                p1 = psC.tile([64, BLK], F32, tag="tmp")
                nc.tensor.matmul(out=p1[:], lhsT=hw["gcn_c1w"][:],
                                 rhs=xp[:, blk_sl(b)], start=True, stop=True)
                a1 = stpool.tile([64, BLK], BF, tag="a1")
                nc.scalar.activation(a1[:], p1[:], AF.Relu,
                                     bias=v(VC["gcn_c1b"], 64), scale=1.0)
                p2 = psC.tile([32, BLK], F32, tag="tmp")
                nc.tensor.matmul(out=p2[:], lhsT=hw["gcn_c2w"][:], rhs=a1[:],
                                 start=True, stop=True)
                a2 = stpool.tile([32, BLK], BF, tag="a2")
                nc.scalar.activation(a2[:], p2[:], AF.Relu,
                                     bias=v(VC["gcn_c2b"], 32), scale=1.0)
                pf = psF.tile([HID, BLK], F32, tag="fuse")
                nc.tensor.matmul(out=pf[:], lhsT=hw["gcn_c3w"][:], rhs=a2[:],
                                 start=True, stop=False, skip_group_check=True)
                p3 = psC.tile([64, BLK], F32, tag="tmp")
                nc.tensor.matmul(out=p3[:], lhsT=hw["gat_c1w"][:],
                                 rhs=xg[:, blk_sl(b)], start=True, stop=True)
                b1 = stpool.tile([64, BLK], BF, tag="a1")
                nc.scalar.activation(b1[:], p3[:], AF.Relu,
                                     bias=v(VC["gat_c1b"], 64), scale=1.0)
                p4 = psC.tile([32, BLK], F32, tag="tmp")
                nc.tensor.matmul(out=p4[:], lhsT=hw["gat_c2w"][:], rhs=b1[:],
                                 start=True, stop=True)
                b2 = stpool.tile([32, BLK], BF, tag="a2")
                nc.scalar.activation(b2[:], p4[:], AF.Relu,
                                     bias=v(VC["gat_c2b"], 32), scale=1.0)
                nc.tensor.matmul(out=pf[:], lhsT=hw["gat_c3w"][:], rhs=b2[:],
                                 start=False, stop=True, skip_group_check=True)
                fs = stpool.tile([HID, BLK], BF, tag="fs")
                nc.scalar.activation(fs[:], pf[:], AF.Identity,
                                     bias=v(VC["fused_b"]), scale=1.0)
                p5 = psC.tile([64, BLK], F32, tag="tmp")
                nc.tensor.matmul(out=p5[:], lhsT=hw["fin_w"][:], rhs=fs[:],
                                 start=True, stop=True)
                f1 = stpool.tile([64, BLK], BF, tag="a1")
                nc.scalar.activation(f1[:], p5[:], AF.Relu,
                                     bias=v(VC["fin_b"], 64), scale=1.0)
                p6 = psC.tile([2, BLK], F32, tag="tmp")
                nc.tensor.matmul(out=p6[:], lhsT=hw["fin2_w"][:], rhs=f1[:],
                                 start=True, stop=True)
                oo = stpool.tile([2, BLK], F32, tag="oo")
                nc.scalar.activation(oo[:], p6[:], AF.Identity,
                                     bias=v(VC["fin2_b"], 2), scale=1.0)
                nc.sync.dma_start(out_fm[:, blk_sl(b)], oo[:])

            # ---- prologue: table 0 from x (AGs asap), then states ----
            for b in range(BPC):
                build_table0_block(b)
                if b == BPC // 2 - 1:
                    ag_half(0, 0)
            ag_half(0, 1)
            for b in range(BPC):
                for (w_sb, bias_col, dst_state) in (
                    (w_in, VC["gcn_in_b"], xp),
                    (w_res, VC["gcn_res_b"], res),
                    (w_gin, VC["gat_in_b"], xg),
                ):
                    p = psC.tile([HID, BLK], F32, tag="tmp")
                    nc.tensor.matmul(out=p[:], lhsT=w_sb[:],
                                     rhs=x_sb[0:IN, blk_sl(b)],
                                     start=True, stop=True)
                    nc.scalar.activation(dst_state[:, blk_sl(b)], p[:], AF.Identity,
                                         bias=v(bias_col), scale=1.0)

            # ================= merged round: GCN + GAT layer i =================
            def round_layer(i):
                h = HEADS[i]
                C = HID // h
                W = 256 + h
                gcol, bcol = VC["gcn_g"][i], VC["gcn_bc"][i]
                gcol2, bcol2 = VC["gat_g"][i], VC["gat_bc"][i]

                # ---- phase A: half-0 sources + self tile -> partial sums ----
                for b in range(BPC):
                    def finishA(acc, b=b):
                        nc.scalar.activation(pacc[:, b, 0:W], acc[:, 0:W],
                                             AF.Copy, bias=0.0, scale=1.0)
                    agg_phase(i, b, tblsA[i], idxA, 0, True, finishA)

                # ---- phase B: half-1 sources, finish, epilogues ----
                for b in range(BPC):
                    def finishB(acc, b=b):
                        fs = stpool.tile([128, 260], F32, tag="facc")
                        nc.vector.tensor_tensor(out=fs[:, 0:W],
                                                in0=acc[:, 0:W],
                                                in1=pacc[:, b, 0:W],
                                                op=ALU.add)
                        # GCN epilogue
                        u = stpool.tile([BLK, HID], BF, tag="gcn_u")
                        nc.scalar.activation(u[:], fs[:, 0:HID], AF.Copy,
                                             bias=0.0, scale=dis[:, b:b + 1])
                        tp = psC.tile([HID, BLK], BF, tag="tmp")
                        nc.tensor.transpose(tp[:], u[:], ident[:])
                        if i == 1:
                            xn = stpool.tile([HID, BLK], BF, tag="gcn_xn")
                            nc.scalar.activation(xn[:], tp[:], AF.Relu,
                                                 bias=v(bcol), scale=v(gcol))
                            nc.vector.tensor_tensor(out=xp[:, blk_sl(b)],
                                                    in0=xn[:],
                                                    in1=res[:, blk_sl(b)],
                                                    op=ALU.add)
                        else:
                            nc.scalar.activation(xp[:, blk_sl(b)], tp[:],
                                                 AF.Relu,
                                                 bias=v(bcol), scale=v(gcol))
                        # GAT epilogue
                        rr = stpool.tile([BLK, 4], F32, tag="rr")
                        nc.vector.tensor_scalar(out=rr[0:BLK, 0:h],
                                                in0=fs[:, 256:256 + h],
                                                scalar1=1e-16, scalar2=None,
                                                op0=ALU.add)
                        nc.vector.reciprocal(rr[0:BLK, 0:h], rr[0:BLK, 0:h])
                        o = stpool.tile([BLK, HID], BF, tag="gat_o")
                        for hh in range(h):
                            nc.scalar.activation(
                                o[:, hh * C:(hh + 1) * C],
                                fs[:, HID + hh * C:HID + (hh + 1) * C],
                                AF.Copy, bias=0.0,
                                scale=rr[0:BLK, hh:hh + 1])
                        tp2 = psC.tile([HID, BLK], BF, tag="tmp")
                        nc.tensor.transpose(tp2[:], o[:], ident[:])
                        # elu(u) = relu(u) - relu(1 - exp(u))
                        r1 = stpool.tile([HID, BLK], BF, tag="gat_r1")
                        q = stpool.tile([HID, BLK], BF, tag="gat_q")
                        r2 = stpool.tile([HID, BLK], BF, tag="gat_r2")
                        nc.scalar.activation(r1[:], tp2[:], AF.Relu,
                                             bias=v(bcol2), scale=v(gcol2))
                        nc.scalar.activation(q[:], tp2[:], AF.Exp,
                                             bias=v(bcol2), scale=v(gcol2))
                        nc.scalar.activation(r2[:], q[:], AF.Relu,
                                             bias=v(VC["ones"]), scale=-1.0)
                        nc.vector.tensor_tensor(out=xg[:, blk_sl(b)],
                                                in0=r1[:], in1=r2[:],
                                                op=ALU.subtract)
                        # next round's table, or final heads
                        if i + 1 < 3:
                            build_table_block(i + 1, b)
                            if b == BPC // 2 - 1:
                                ag_half(i + 1, 0)
                        else:
                            do_heads(b)
                    agg_phase(i, b, tblsB[i], idxB, TTA, False, finishB)
                if i + 1 < 3:
                    ag_half(i + 1, 1)

            for i in range(min(ngcn, 3)):
                round_layer(i)

    nc.compile()
    return nc


# ----------------------------------------------------------------------------
# Host orchestration
# ----------------------------------------------------------------------------

def make_inputs_per_core(inputs, src_pads, dst_pad, dstloc, dis, T_half):
    srcA, srcB = src_pads
    TTT = 2 * T_half + 1
    TILA = BPC * TTT

    x = np.asarray(inputs["x"], np.float32)
    xpad = np.zeros((NPAD, IN + 1), np.float32)
    xpad[:N, :IN] = x
    xpad[:, IN] = 1.0

    g = lambda k: np.asarray(inputs[k], np.float32)

    vecs = np.zeros((128, 32), np.float32)

    def setv(col, arr):
        arr = np.asarray(arr, np.float32).ravel()
        vecs[: arr.shape[0], col] = arr

    VC = dict(gcn_in_b=0, gcn_res_b=1, gat_in_b=2,
              gcn_g=[3, 4, 5], gcn_bc=[6, 7, 8],
              gat_g=[9, 10, 11], gat_bc=[12, 13, 14],
              gcn_c1b=15, gcn_c2b=16, fused_b=17,
              gat_c1b=18, gat_c2b=19, fin_b=20, fin2_b=21, ones=22)
    setv(VC["gcn_in_b"], g("gcn_in_b"))
    setv(VC["gcn_res_b"], g("gcn_res_b"))
    setv(VC["gat_in_b"], g("gat_in_b"))
    for i in range(3):
        gp = bn_fold(g("gcn_bn_g")[i])
        setv(VC["gcn_g"][i], gp)
        setv(VC["gcn_bc"][i], g("gcn_b")[i] * gp + g("gcn_bn_b")[i])
        gp2 = bn_fold(g("gat_bn_g")[i])
        setv(VC["gat_g"][i], gp2)
        setv(VC["gat_bc"][i], g("gat_b")[i] * gp2 + g("gat_bn_b")[i])
    setv(VC["gcn_c1b"], g("gcn_c1b"))
    setv(VC["gcn_c2b"], g("gcn_c2b"))
    setv(VC["fused_b"], 0.6 * g("gcn_c3b") + 0.4 * g("gat_c3b"))
    setv(VC["gat_c1b"], g("gat_c1b"))
    cg = bn_fold(g("gat_cbn_g"))
    setv(VC["gat_c2b"], g("gat_cbn_b") @ g("gat_c2w") + g("gat_c2b"))
    setv(VC["fin_b"], g("fin_b"))
    fg = bn_fold(g("fin_bn_g"))
    setv(VC["fin2_b"], g("fin_bn_b") @ g("fin2_w") + g("fin2_b"))
    setv(VC["ones"], np.ones(128, np.float32))

    gcn_c3wf = 0.6 * g("gcn_c3w")
    gat_c1wf = g("feat_imp")[:, None] * g("gat_c1w")
    gat_c2wf = cg[:, None] * g("gat_c2w")
    gat_c3wf = 0.4 * g("gat_c3w")
    fin2_wf = fg[:, None] * g("fin2_w")

    gat_wgs = []
    for i in range(3):
        W = g("gat_w")[i]
        asrc_bd = make_blockdiag(g("gat_asrc")[i], HEADS[i])
        adst_bd = make_blockdiag(g("gat_adst")[i], HEADS[i])
        gat_wgs.append(np.concatenate([W, W @ asrc_bd, W @ adst_bd], axis=1)
                       .astype(np.float32))

    t0g = np.vstack([g("gcn_in_w") @ g("gcn_w")[0],
                     g("gcn_in_b") @ g("gcn_w")[0]])
    t0a = np.vstack([g("gat_in_w") @ gat_wgs[0],
                     g("gat_in_b") @ gat_wgs[0]])

    iota = np.tile(np.arange(128, dtype=np.int8)[None, :], (128, T_half + 1))
    iotac = np.arange(128, dtype=np.float32).reshape(128, 1)

    in_maps = []
    for c in range(NC):
        b0 = c * BPC
        nodes = slice(b0 * BLK, (b0 + BPC) * BLK)
        sgA = srcA[b0:b0 + BPC].reshape(-1, 128)      # relative tblA rows
        sgB = srcB[b0:b0 + BPC].reshape(-1, 128)
        dloc = dstloc[b0:b0 + BPC].reshape(-1, 128)   # [TILA, 128] int32
        im = dict(
            x_fm=np.ascontiguousarray(xpad[nodes].T).astype(NPBF),
            idx_a=wrap_idx_core(sgA),
            idx_b=wrap_idx_core(sgB),
            dstloc=np.ascontiguousarray(dloc.T).astype(np.int8),
            dlt=np.broadcast_to(dloc.reshape(1, -1).astype(np.int8),
                    (128, TILA * 128)).copy(),
            iota_rep=iota,
            iotac=iotac,
            dis=np.ascontiguousarray(dis[nodes].reshape(BPC, BLK).T),
            t0g=t0g.astype(NPBF), t0a=t0a.astype(NPBF),
            gcn_in_w=g("gcn_in_w").astype(NPBF),
            gcn_res_w=g("gcn_res_w").astype(NPBF),
            gat_in_w=g("gat_in_w").astype(NPBF),
            gcn_w=g("gcn_w").astype(NPBF),
            gat_wg0=gat_wgs[0].astype(NPBF), gat_wg1=gat_wgs[1].astype(NPBF),
            gat_wg2=gat_wgs[2].astype(NPBF),
            vecs=vecs,
            gcn_c1w=g("gcn_c1w").astype(NPBF),
            gcn_c2w=g("gcn_c2w").astype(NPBF),
            gcn_c3w=gcn_c3wf.astype(NPBF),
            gat_c1w=gat_c1wf.astype(NPBF),
            gat_c2w=gat_c2wf.astype(NPBF),
            gat_c3w=gat_c3wf.astype(NPBF),
            fin_w=g("fin_w").astype(NPBF),
            fin2_w=fin2_wf.astype(NPBF),
        )
        in_maps.append(im)
    return in_maps


_CACHE = {}


def _get_program(T_half):
    if T_half not in _CACHE:
        _CACHE[T_half] = build_program(T_half)
    return _CACHE[T_half]


def kernel(**inputs):
    edge_index = np.asarray(inputs["edge_index"])
    src_pads, dst_pad, dstloc, dis, T_half = preprocess_graph(edge_index)
    in_maps = make_inputs_per_core(inputs, src_pads, dst_pad, dstloc, dis,
                                   T_half)
    nc = _get_program(T_half)
    res = run_bass_kernel_spmd(nc, in_maps, core_ids=list(range(NC)))
    out = np.concatenate([res.results[c]["out_fm"].T for c in range(NC)], axis=0)
    return np.ascontiguousarray(out[:N]).astype(np.float32)
